# revision 59
# baseline (speedup 1.0000x reference)
"""AttentionBlock (GroupNorm + 1x1-conv QKV + full NxN attention + proj +
residual) on 8 Trainium2 NeuronCores, data-parallel over the batch dim.

Per core: 2 samples of x[16, 512, 32, 32]. Matmul operands are fp8e4m3
with DoubleRow perf mode (2 contraction planes per instruction - the
128x128 PE array virtualizes to 128x256). PSUM accumulation stays fp32;
the residual path is bf16 (x is converted to bf16 on the host, halving
input DMA, and the output is written bf16 and upcast on the host).

Algebraic folds done on the host (exact):
  - GroupNorm affine (norm_w, norm_b) folded into qkv_w / qkv_b.
  - K bias dropped entirely: S_ji += bk.Q_i is constant along the
    softmax axis j, so it cancels between numerator and denominator.
  - V bias and proj bias folded into one per-channel bias
    pb' = proj_w @ bv + proj_b  (softmax rows sum to 1).
  - softmax denominator folded through the projection:
    out = (x + pb') + (proj_w @ (V @ E^T)) * (1/Z).

Performance structure:
  - All matmul PSUM goes through two-bank pair tiles [128,2,512] so
    evacuations are one wide op per two matmul tiles (halves ACT/DVE
    per-instruction overhead and PSUM-bank grab events); the exp of
    S^T is a single [128,2,512] ACT op per pair.
  - GroupNorm stats are split ACT (tile 0 via activation+accum) / DVE
    (tiles 1-3 via bn_stats, staging smalls on the idle GPSIMD) and the
    normalize applies are split DVE/ACT (vals carry -mean*rstd so both
    engines express x*rstd - mean*rstd), cutting startup latency; x
    arrives as half-tile descriptors over three DMA queues.
  - Input DMA is two waves: wave A (sample 0's x, the Q weights, the
    one-hot constants) goes immediately; wave B (remaining weights,
    biases, sample 1's x) is gated behind wave A's last x tile by tiny
    WAW dep-copies so it cannot steal HBM bandwidth from the critical
    path, with descriptors posted from the idle sync queue.
  - Phase order interleaves the two samples: qkv0, S0 (s1's GroupNorm
    rides inside), qkv1, S1a, attn-tail(s0,ib0), S1b, attn-tail(s0,ib1),
    attn-tail(s1,*). Each phase's Z row-sum + 1/Z = exp(-ln Z) chain and
    its partition broadcast (zb) are hoisted at least one phase ahead of
    the O evacuations that consume them, so the PE and DVE never wait on
    the ACT ln/exp chain. The final i-block's output DMA is split per
    channel tile across three queues to shorten the drain.
"""

import math
import sys

import numpy as np

try:
    import concourse.bass as bass
except ImportError:  # pragma: no cover - grading container path setup
    sys.path.insert(0, "/opt/trn_rl_repo")
    import concourse.bass as bass

import bass_rust
import ml_dtypes
import concourse.tile as tile
from concourse import mybir
from concourse.bass_utils import run_bass_kernel_spmd

F32 = mybir.dt.float32
BF16 = mybir.dt.bfloat16
FP8 = mybir.dt.float8e4
DR = mybir.MatmulPerfMode.DoubleRow
AF = mybir.ActivationFunctionType
OP = mybir.AluOpType

NCORES = 8
B = 16
S = B // NCORES  # samples per core
C = 512
N = 1024  # H*W
G = 8  # groups
EPS = 1e-5
CT = C // 128  # channel p-tiles (4)
NT = N // 128  # spatial p-tiles (8)
IBS = 512  # i-block size
IB = N // IBS  # i blocks (2)
INV_SQRT_C = 1.0 / math.sqrt(C)

# Settable by test harness for profiling; not used by the grader.
TRACE = False
LAST_RESULT = None


MAX_WAITS = 1


def _split_excess_waits(nc, max_waits=MAX_WAITS):
    """Workaround for a walrus codegen limit: an instruction may carry at
    most `max_waits` semaphore waits ("Too many sync wait commands").
    Move the excess onto a chain of NOPs on the same engine right before
    the instruction - sequentially blocking waits on one engine queue are
    semantically identical to one multi-wait instruction."""
    counter = 0
    for f in nc.m.functions:
        for blk in f.blocks:
            il = blk.instructions
            if not any(
                i.sync_info is not None and len(i.sync_info.on_wait) > max_waits
                for i in il
            ):
                continue
            old = list(il)
            il.clear()
            for ins in old:
                si = ins.sync_info
                waits = list(si.on_wait) if si is not None else []
                if len(waits) > max_waits:
                    excess, keep = waits[:-max_waits], waits[-max_waits:]
                    for i0 in range(0, len(excess), max_waits):
                        counter += 1
                        nop = mybir.InstNoOp(
                            name=f"waitsplit-{counter}",
                            engine=ins.engine,
                            ins=[],
                            outs=[],
                            sync_info=bass_rust.SyncInfo(
                                on_wait=excess[i0 : i0 + max_waits], on_update=[]
                            ),
                        )
                        nc.register_instruction(nop, overwrite=True)
                        blk.add_instruction(nop)
                    ins.sync_info = bass_rust.SyncInfo(
                        on_wait=keep, on_update=list(si.on_update)
                    )
                blk.add_instruction(ins)
    return counter


def _build():
    from contextlib import ExitStack

    nc = bass.Bass()
    xd = nc.declare_dram_parameter("x", [S, C, N], BF16, isOutput=False)
    wald = nc.declare_dram_parameter("wall", [128, 4 * CT, C], FP8, isOutput=False)
    bald = nc.declare_dram_parameter("ball", [128, 2 * CT], F32, isOutput=False)
    cad = nc.declare_dram_parameter("ca", [128, 2], BF16, isOutput=False)
    cbd = nc.declare_dram_parameter("cb", [2, 128], BF16, isOutput=False)
    outd = nc.declare_dram_parameter("out", [S, C, N], BF16, isOutput=True)

    x_ap = xd[:].rearrange("s (t p) n -> s t p n", p=128)
    out_ap = outd[:].rearrange("s (t p) n -> s p t n", p=128)

    with tile.TileContext(nc) as tc, ExitStack() as ctx:
        singles = ctx.enter_context(tc.tile_pool(name="singles", bufs=1))
        xp = ctx.enter_context(tc.tile_pool(name="xp", bufs=S))
        xnp = ctx.enter_context(tc.tile_pool(name="xnp", bufs=S))
        qp = ctx.enter_context(tc.tile_pool(name="qp", bufs=2))
        kp = ctx.enter_context(tc.tile_pool(name="kp", bufs=2))
        vp = ctx.enter_context(tc.tile_pool(name="vp", bufs=2))
        ep = ctx.enter_context(tc.tile_pool(name="ep", bufs=16))
        op_ = ctx.enter_context(tc.tile_pool(name="op", bufs=3))
        rzp = ctx.enter_context(tc.tile_pool(name="rzp", bufs=6))
        obp = ctx.enter_context(tc.tile_pool(name="obp", bufs=4))
        smp = ctx.enter_context(tc.tile_pool(name="smp", bufs=4))
        ps = ctx.enter_context(tc.tile_pool(name="ps", bufs=1, space="PSUM"))

        # ---- wave-A input DMAs: sample 0's x, wq, one-hot constants ----
        wall = singles.tile([128, 4 * CT, C], FP8)
        ball = singles.tile([128, 2 * CT], F32)
        ca = singles.tile([128, 2], BF16)
        cb = singles.tile([2, 128], BF16)
        xs = []
        for s in range(S):
            x_sb = xp.tile([128, CT, N], BF16, tag="x", name=f"x{s}")
            xs.append(x_sb)
        # dummy activation at the head of the ACT queue so the ~1.3us
        # activation-table load runs during the DMA ramp, not after it
        dummy = singles.tile([1, 1], F32)
        nc.scalar.memzero(dummy[:])
        nc.scalar.activation(out=dummy[:], in_=dummy[:], func=AF.Exp)
        # half-tile descriptors so stats can start on the first 512 columns
        # while the rest streams in; three queues post in parallel so tiles
        # t0/t1/t2 complete together, then t3
        for t, eng in ((0, nc.sync), (1, nc.gpsimd), (2, nc.scalar),
                       (3, nc.scalar)):
            eng.dma_start(out=xs[0][:, t, 0:512], in_=x_ap[0, t][:, 0:512])
            eng.dma_start(out=xs[0][:, t, 512:1024], in_=x_ap[0, t][:, 512:1024])
        nc.sync.dma_start(out=wall[:, 0:CT, :], in_=wald[:, 0:CT, :])
        nc.gpsimd.dma_start(out=ca[:], in_=cad[:])
        nc.gpsimd.dma_start(out=cb[:], in_=cbd[:])
        eps_sb = singles.tile([2, 1], F32)
        nc.vector.memset(eps_sb[:], EPS)
        ones8 = singles.tile([128, 2, 16], FP8)
        nc.vector.memset(ones8[:], 1.0)
        ones_row = singles.tile([1, 128], BF16)
        nc.vector.memset(ones_row[:], 1.0)


        # ---- wave-B DMAs, gated behind wave A's last x tile: tiny WAW
        # dep-copies write a few bytes into each wave-B destination, so
        # every wave-B DMA write-after-write depends on x tile 3. Emitted
        # (priority-wise) after sample 0's stats so the gpsimd staging
        # smalls are not delayed; correctness rides on the data deps. ----
        def emit_waveb():
            gate = xs[0][:, 3, 512:513]
            for dst in (
                xs[1][:, 0, 0:1],
                xs[1][:, 1, 0:1],
                xs[1][:, 2, 0:1],
                xs[1][:, 3, 0:1],
            ):
                nc.gpsimd.tensor_copy(out=dst, in_=gate)
            gate8 = singles.tile([128, 1], FP8, name="gate8")
            nc.gpsimd.tensor_copy(out=gate8[:], in_=gate)
            for wi in range(1, 4):
                nc.gpsimd.tensor_copy(out=wall[:, wi * CT, 0:1], in_=gate8[:])
            gatef = singles.tile([128, 1], F32, name="gatef")
            nc.gpsimd.tensor_copy(out=gatef[:], in_=gate)
            nc.gpsimd.tensor_copy(out=ball[:, 0:1], in_=gatef[:])
            # wave-B descriptors ride the sync queue (idle after wave A) so
            # the gpsimd queue stays free for the GroupNorm staging smalls
            for t in range(CT):
                nc.sync.dma_start(out=xs[1][:, t, :], in_=x_ap[1, t])
            for wi in range(1, 4):
                nc.sync.dma_start(
                    out=wall[:, wi * CT : (wi + 1) * CT, :],
                    in_=wald[:, wi * CT : (wi + 1) * CT, :],
                )
            nc.sync.dma_start(out=ball[:], in_=bald[:])

        # weight planes for DoubleRow: [p, wi, g, q, o]; channel = 256g+128q+p
        w8 = wall.rearrange("p (w g q) f -> p w g q f", g=2, q=2)
        bq, pb = (ball[:, i * CT : (i + 1) * CT] for i in range(2))
        ghot, hhot = ca[:, 0:2], cb[:, :]

        xns = [None] * S
        sa_hs = [None] * S
        qkvs = [None] * S
        es_all = [[None] * IB for _ in range(S)]
        rzs = [[None] * IB for _ in range(S)]

        def emit_gn_alloc(s):
            xns[s] = [
                xnp.tile([128, 2, N], FP8, tag="xn", bufs=2 * S, name=f"xn{s}{g}")
                for g in range(2)
            ]
            sa_hs[s] = [
                smp.tile([128, 4], BF16, tag="sa", name=f"sa{s}{h}")
                for h in range(2)
            ]

        scr = singles.tile([128, N], BF16, name="scr")

        def emit_gn_stats_act(s, tiles):
            """ACT stats path (activation + free-dim accumulate); runs in
            parallel with the DVE bn_stats path to cut startup latency."""
            x_sb = xs[s]
            for t in tiles:
                sa = sa_hs[s][t // 2]
                c0 = 2 * (t % 2)
                sum3 = smp.tile([128, 1], F32, tag="sum3")
                nc.scalar.activation(
                    out=scr[:], in_=x_sb[:, t, :], func=AF.Identity,
                    accum_out=sum3[:],
                )
                sq3 = smp.tile([128, 1], F32, tag="sq3")
                nc.scalar.activation(
                    out=scr[:], in_=x_sb[:, t, :], func=AF.Square,
                    accum_out=sq3[:],
                )
                nc.scalar.activation(
                    out=sa[:, c0 : c0 + 1], in_=sum3[:], func=AF.Identity,
                    scale=1.0 / N,
                )
                nc.scalar.activation(
                    out=sa[:, c0 + 1 : c0 + 2], in_=sq3[:], func=AF.Identity,
                    scale=1.0 / N,
                )

        def emit_gn_stats(s, tiles):
            """DVE bn_stats; the [mean, E[x^2]] staging smalls ride on the
            otherwise-idle GPSIMD engine (fast at [128,1] sizes) so the DVE
            serial chain is just bn_stats + bn_aggr."""
            x_sb = xs[s]
            for t in tiles:
                sa = sa_hs[s][t // 2]
                c0 = 2 * (t % 2)
                st6 = smp.tile([128, 2, 6], F32, tag="st6")
                nc.vector.bn_stats(out=st6[:, 0, :], in_=x_sb[:, t, 0:512])
                nc.vector.bn_stats(out=st6[:, 1, :], in_=x_sb[:, t, 512:1024])
                mv = smp.tile([128, 2], F32, tag="mv")
                nc.vector.bn_aggr(out=mv[:], in_=st6[:])
                nc.gpsimd.tensor_copy(out=sa[:, c0 : c0 + 1], in_=mv[:, 0:1])
                msq = smp.tile([128, 1], F32, tag="msq")
                nc.gpsimd.tensor_mul(msq[:], mv[:, 0:1], mv[:, 0:1])
                nc.gpsimd.tensor_tensor(
                    out=sa[:, c0 + 1 : c0 + 2],
                    in0=mv[:, 1:2],
                    in1=msq[:],
                    op=OP.add,
                )

        bcss = [[None, None] for _ in range(S)]

        def emit_gn_chain(s, h0):
            """group reduce -> rstd -> broadcast for tile pair h0."""
            gs_ps = ps.tile([2, 4], F32, tag="zbp", bufs=1, name=f"gs{s}{h0}")
            nc.tensor.matmul(
                gs_ps[:], lhsT=ghot, rhs=sa_hs[s][h0][:], start=True, stop=True
            )
            gs3 = gs_ps.rearrange("h (t s) -> h t s", s=2)
            sq = smp.tile([2, 2], F32, tag="sq")
            nc.scalar.activation(out=sq[:], in_=gs3[:, :, 0], func=AF.Square)
            var = smp.tile([2, 2], F32, tag="var")
            nc.vector.tensor_tensor(
                out=var[:], in0=gs3[:, :, 1], in1=sq[:], op=OP.subtract
            )
            lnv = smp.tile([2, 2], F32, tag="lnv")
            nc.scalar.activation(
                out=lnv[:], in_=var[:], func=AF.Ln, bias=eps_sb[:], scale=1.0
            )
            # vals: (rstd, -mean*rstd); apply is x*rstd + (-mean*rstd),
            # expressible on DVE (tensor_scalar) and ACT (scale/bias)
            vals = smp.tile([2, 4], BF16, tag="vals")
            vals3 = vals.rearrange("h (t s) -> h t s", s=2)
            nc.scalar.activation(
                out=vals3[:, :, 0], in_=lnv[:], func=AF.Exp, scale=-0.5
            )
            nc.vector.scalar_tensor_tensor(
                out=vals3[:, :, 1], in0=gs3[:, :, 0], scalar=-1.0,
                in1=vals3[:, :, 0], op0=OP.mult, op1=OP.mult,
            )
            bc = ps.tile([128, 4], F32, tag="zbp", bufs=1, name=f"bc{s}{h0}")
            nc.tensor.matmul(bc[:], lhsT=hhot, rhs=vals[:], start=True, stop=True)
            bcs = smp.tile([128, 4], F32, tag="bcs")
            nc.vector.tensor_copy(out=bcs[:], in_=bc[:])
            bcss[s][h0] = bcs

        def emit_gn_apply(s, h0, engines=("v", "v")):
            bcs = bcss[s][h0]
            for tt in range(2):
                t = 2 * h0 + tt
                if engines[tt] == "v":
                    nc.vector.tensor_scalar(
                        out=xns[s][h0][:, tt, :],
                        in0=xs[s][:, t, :],
                        scalar1=bcs[:, 2 * tt : 2 * tt + 1],
                        scalar2=bcs[:, 2 * tt + 1 : 2 * tt + 2],
                        op0=OP.mult,
                        op1=OP.add,
                    )
                else:  # ACT: out = Identity(x*rstd + (-mean*rstd))
                    nc.scalar.activation(
                        out=xns[s][h0][:, tt, :],
                        in_=xs[s][:, t, :],
                        func=AF.Identity,
                        scale=bcs[:, 2 * tt : 2 * tt + 1],
                        bias=bcs[:, 2 * tt + 1 : 2 * tt + 2],
                    )

        def emit_qkv(s):
            xn_g = xns[s]
            q_sb = qp.tile([128, 2, 2, N], FP8, tag="q")
            k_sb = kp.tile([128, 2, 2, N], FP8, tag="k")
            v_sb = vp.tile([128, NT // 2, 2, C], FP8, tag="v")
            qkvs[s] = (q_sb, k_sb, v_sb)
            qv = q_sb.rearrange("p g q n -> p (g q) n")
            kv = k_sb.rearrange("p g q n -> p (g q) n")
            # per-tile (half-used pair slot) PSUM so each evacuation
            # depends only on its own matmuls, not a whole merged pair
            for wi, dst in ((0, qv), (1, kv)):
                for ot in range(CT):
                    for ib in range(IB):
                        psm = ps.tile([128, 2, IBS], F32, tag="mmp", bufs=3)
                        for g in range(2):
                            nc.tensor.matmul(
                                psm[:, 0, :],
                                lhsT=w8[:, wi, g, :, ot * 128 : (ot + 1) * 128],
                                rhs=xn_g[g][:, :, ib * IBS : (ib + 1) * IBS],
                                start=(g == 0),
                                stop=(g == 1),
                                perf_mode=DR,
                            )
                        if wi == 0:
                            # Q bias folded into the DVE evacuation
                            nc.vector.tensor_scalar(
                                out=dst[:, ot, ib * IBS : (ib + 1) * IBS],
                                in0=psm[:, 0, :],
                                scalar1=bq[:, ot : ot + 1],
                                scalar2=None,
                                op0=OP.add,
                            )
                        else:
                            # K bias dropped (cancels in softmax): plain cast
                            nc.vector.tensor_copy(
                                out=dst[:, ot, ib * IBS : (ib + 1) * IBS],
                                in_=psm[:, 0, :],
                            )
            for nt in range(NT):
                psm = ps.tile([128, 2, IBS], F32, tag="mmp", bufs=3)
                for g in range(2):
                    nc.tensor.matmul(
                        psm[:, 0, :],
                        lhsT=xn_g[g][:, :, nt * 128 : (nt + 1) * 128],
                        rhs=w8[:, 2, g, :, :],
                        start=(g == 0),
                        stop=(g == 1),
                        perf_mode=DR,
                    )
                # V evacuation on ACT (DVE is the busier engine here)
                nc.scalar.activation(
                    out=v_sb[:, nt // 2, nt % 2, :], in_=psm[:, 0, :],
                    func=AF.Identity,
                )

        def emit_S(s, ibs):
            """S^T matmuls + one wide exp per pair of j-tiles."""
            q_sb, k_sb, _ = qkvs[s]
            for ib in ibs:
                isl = slice(ib * IBS, (ib + 1) * IBS)
                es = []
                es_all[s][ib] = es
                for jp in range(NT // 2):
                    psm = ps.tile([128, 2, IBS], F32, tag="mmp", bufs=3)
                    for i in range(2):
                        jt = 2 * jp + i
                        for g in range(2):
                            nc.tensor.matmul(
                                psm[:, i, :],
                                lhsT=k_sb[:, g, :, jt * 128 : (jt + 1) * 128],
                                rhs=q_sb[:, g, :, isl],
                                start=(g == 0),
                                stop=(g == 1),
                                perf_mode=DR,
                            )
                    e = ep.tile([128, 2, IBS], FP8, tag="e")
                    es.append(e)
                    nc.scalar.activation(
                        out=e[:], in_=psm[:], func=AF.Exp, scale=INV_SQRT_C
                    )

        def emit_Z(s, ib):
            """Z row-sums + the ACT ln/exp chain for 1/Z; emitted one phase
            early so the zb broadcast never waits on ACT."""
            es = es_all[s][ib]
            zps = ps.tile([1, IBS], F32, tag="zrow", bufs=1, name=f"z{s}{ib}")
            for jg in range(NT // 2):
                nc.tensor.matmul(
                    zps[:],
                    lhsT=ones8[:, :, 0:1],
                    rhs=es[jg][:],
                    start=(jg == 0),
                    stop=(jg == NT // 2 - 1),
                    perf_mode=DR,
                )
            lnz = rzp.tile([1, IBS], F32, tag="lnz")
            nc.scalar.activation(out=lnz[:], in_=zps[:], func=AF.Ln)
            rz = rzp.tile([1, IBS], BF16, tag="rz")
            nc.scalar.activation(out=rz[:], in_=lnz[:], func=AF.Exp, scale=-1.0)
            rzs[s][ib] = rz

        zbs = [[None] * IB for _ in range(S)]

        def emit_zb(s, ib):
            """1/Z broadcast across partitions into SBUF, hoisted well ahead
            of the O evacuations that read it."""
            zb_ps = ps.tile([128, IBS], F32, tag="zbp", bufs=1,
                            name=f"zbp{s}{ib}")
            nc.tensor.matmul(
                zb_ps[:], lhsT=ones_row, rhs=rzs[s][ib][:], start=True,
                stop=True,
            )
            zb = obp.tile([128, IBS], F32, tag="zbs", bufs=4, name=f"zb{s}{ib}")
            nc.vector.tensor_copy(out=zb[:], in_=zb_ps[:])
            zbs[s][ib] = zb

        def emit_attn2(s, ib):
            """O = V E^T, proj, residual evac, output DMA."""
            x_sb = xs[s]
            _, _, v_sb = qkvs[s]
            isl = slice(ib * IBS, (ib + 1) * IBS)
            es = es_all[s][ib]
            o_sb = op_.tile([128, 2, 2, IBS], FP8, tag="o")
            ov = o_sb.rearrange("p g q n -> p (g q) n")
            zb = zbs[s][ib]
            # one (pair-shaped, half-used) PSUM tile per ct so each
            # evacuation depends only on its own 4 matmuls - group tracking
            # is tile-granular, so sharing a tile across two ct's made the
            # first evac wait for both halves and stalled the PE each phase
            for ct in range(CT):
                psm = ps.tile([128, 2, IBS], F32, tag="mmp", bufs=3)
                for jg in range(NT // 2):
                    nc.tensor.matmul(
                        psm[:, 0, :],
                        lhsT=v_sb[:, jg, :, ct * 128 : (ct + 1) * 128],
                        rhs=es[jg][:],
                        start=(jg == 0),
                        stop=(jg == NT // 2 - 1),
                        perf_mode=DR,
                    )
                nc.vector.tensor_tensor(
                    out=ov[:, ct, :], in0=psm[:, 0, :], in1=zb[:], op=OP.mult
                )
            ob4 = obp.tile([128, CT, IBS], BF16, tag="ob", bufs=3)
            for ot in range(CT):
                psm = ps.tile([128, 2, IBS], F32, tag="mmp", bufs=3)
                for g in range(2):
                    nc.tensor.matmul(
                        psm[:, 0, :],
                        lhsT=w8[:, 3, g, :, ot * 128 : (ot + 1) * 128],
                        rhs=o_sb[:, g, :, :],
                        start=(g == 0),
                        stop=(g == 1),
                        perf_mode=DR,
                    )
                # out = (psum + pb') + x in one pass
                nc.vector.scalar_tensor_tensor(
                    out=ob4[:, ot, :],
                    in0=psm[:, 0, :],
                    scalar=pb[:, ot : ot + 1],
                    in1=x_sb[:, ot, isl],
                    op0=OP.add,
                    op1=OP.add,
                )
                if s == S - 1 and ib == IB - 1:
                    # tail: one DMA per ot, spread across queues so the
                    # descriptor issue doesn't serialize the drain
                    eng = (nc.sync, nc.scalar, nc.gpsimd, nc.sync)[ot]
                    eng.dma_start(
                        out=out_ap[s][:, ot : ot + 1, isl],
                        in_=ob4[:, ot : ot + 1, :],
                    )
            if not (s == S - 1 and ib == IB - 1):
                nc.sync.dma_start(out=out_ap[s][:, :, isl], in_=ob4[:])

        # ---- software-pipelined emission across the two samples ----
        emit_gn_alloc(0)
        emit_gn_stats_act(0, [0])   # t0 on ACT, in parallel with DVE
        emit_gn_stats(0, [1, 2])
        with tc.high_priority():
            emit_gn_chain(0, 0)
            emit_gn_apply(0, 0, ("v", "s"))  # t1's apply rides on ACT
        emit_gn_stats(0, [3])
        emit_waveb()
        with tc.high_priority():
            emit_gn_chain(0, 1)
            emit_gn_apply(0, 1, ("v", "s"))
        emit_qkv(0)
        emit_S(0, [0])          # s0 S phase, first i-block
        emit_gn_alloc(1)
        emit_gn_stats(1, [0, 1, 2, 3])  # s1 stats fill idle DVE here
        # high priority: slot the s1 chain/applies in as soon as their deps
        # are ready (the scheduler otherwise defers them until right before
        # qkv1 and the cross-engine ping-pong stalls the PE there)
        with tc.high_priority():
            emit_gn_chain(1, 0)
            emit_gn_chain(1, 1)
        emit_S(0, [1])          # s0 S phase, second i-block
        with tc.high_priority():
            emit_gn_apply(1, 0)
            emit_gn_apply(1, 1)
        emit_qkv(1)             # fills the PE while s0's exps drain
        emit_Z(0, 0)
        emit_S(1, [0])          # s1's first exps drain under attn2(0,0)
        emit_zb(0, 0)
        emit_attn2(0, 0)
        emit_Z(0, 1)
        emit_S(1, [1])
        emit_Z(1, 0)            # s1 Z heads + all remaining zb broadcasts
        emit_zb(0, 1)           # early: the tail phases then have no ACT
        emit_Z(1, 1)            # dependencies at all
        emit_zb(1, 0)
        emit_attn2(0, 1)
        emit_zb(1, 1)
        emit_attn2(1, 0)
        emit_attn2(1, 1)

    _split_excess_waits(nc)
    return nc


_NC = None


def kernel(x, norm_w, norm_b, qkv_w, qkv_b, proj_w, proj_b):
    global _NC, LAST_RESULT
    x = np.asarray(x, dtype=np.float32)
    norm_w = np.asarray(norm_w, dtype=np.float32)
    norm_b = np.asarray(norm_b, dtype=np.float32)
    qkv_w = np.asarray(qkv_w, dtype=np.float32)
    qkv_b = np.asarray(qkv_b, dtype=np.float32)
    proj_w = np.asarray(proj_w, dtype=np.float32)
    proj_b = np.asarray(proj_b, dtype=np.float32)

    # fold GroupNorm affine into qkv
    wq_full = qkv_w * norm_w[None, :]
    bq_full = qkv_b + qkv_w @ norm_b
    wq_, wk_, wv_ = wq_full[0:C], wq_full[C : 2 * C], wq_full[2 * C : 3 * C]
    bq_, bv_ = bq_full[0:C], bq_full[2 * C : 3 * C]
    pb_ = proj_w @ bv_ + proj_b

    def wtile(w):  # [o, c] -> DoubleRow lhsT planes [128, 2(g), 2(q), o]
        return w.T.reshape(2, 2, 128, C).transpose(2, 0, 1, 3)

    def btile(b):  # [C] -> [128, ct]
        return b.reshape(CT, 128).T

    wall = np.ascontiguousarray(
        np.stack(
            [wtile(wq_), wtile(wk_), wtile(wv_), wtile(proj_w)], axis=1
        ).reshape(128, 16, C).astype(ml_dtypes.float8_e4m3)
    )
    ball = np.ascontiguousarray(
        np.concatenate([btile(bq_), btile(pb_)], axis=1).astype(np.float32)
    )
    cl = np.arange(128)
    ghot = np.zeros((128, 2), np.float32)
    ghot[cl, cl // 64] = 1.0 / 64.0
    hhot = np.zeros((2, 128), np.float32)
    hhot[cl // 64, cl] = 1.0

    common = {
        "wall": wall,
        "ball": ball,
        "ca": ghot.astype(ml_dtypes.bfloat16),
        "cb": hhot.astype(ml_dtypes.bfloat16),
    }
    xr = np.ascontiguousarray(
        x.reshape(NCORES, S, C, N).astype(ml_dtypes.bfloat16)
    )
    in_maps = [dict(common, x=xr[i]) for i in range(NCORES)]

    if _NC is None:
        _NC = _build()
    res = run_bass_kernel_spmd(
        _NC, in_maps, core_ids=list(range(NCORES)), trace=TRACE
    )
    LAST_RESULT = res
    out = np.stack([res.results[i]["out"] for i in range(NCORES)])
    return np.ascontiguousarray(
        out.reshape(B, C, 32, 32).astype(np.float32)
    )


# revision 60
# speedup vs baseline: 1.1666x; 1.1666x over previous
"""AttentionBlock (GroupNorm + 1x1-conv QKV + full NxN attention + proj +
residual) on 8 Trainium2 NeuronCores, data-parallel over the batch dim.

Per core: 2 samples of x[16, 512, 32, 32]. Matmul operands are fp8e4m3
with DoubleRow perf mode (2 contraction planes per instruction - the
128x128 PE array virtualizes to 128x256). PSUM accumulation stays fp32;
the residual path is bf16 (x is converted to bf16 on the host, halving
input DMA, and the output is written bf16 and upcast on the host).

Algebraic folds done on the host (exact):
  - GroupNorm affine (norm_w, norm_b) folded into qkv_w / qkv_b.
  - K bias dropped entirely: S_ji += bk.Q_i is constant along the
    softmax axis j, so it cancels between numerator and denominator.
  - V bias and proj bias folded into one per-channel bias
    pb' = proj_w @ bv + proj_b  (softmax rows sum to 1).
  - softmax denominator folded through the projection:
    out = (x + pb') + (proj_w @ (V @ E^T)) * (1/Z).

Performance structure:
  - All matmul PSUM goes through two-bank pair tiles [128,2,512] so
    evacuations are one wide op per two matmul tiles (halves ACT/DVE
    per-instruction overhead and PSUM-bank grab events); the exp of
    S^T is a single [128,2,512] ACT op per pair.
  - GroupNorm stats are split ACT (tile 0 via activation+accum) / DVE
    (tiles 1-3 via bn_stats, staging smalls on the idle GPSIMD) and the
    normalize applies are split DVE/ACT (vals carry -mean*rstd so both
    engines express x*rstd - mean*rstd), cutting startup latency; x
    arrives as half-tile descriptors over three DMA queues.
  - Input DMA is two waves: wave A (sample 0's x, the Q weights, the
    one-hot constants) goes immediately; wave B (remaining weights,
    biases, sample 1's x) is gated behind wave A's last x tile by tiny
    WAW dep-copies so it cannot steal HBM bandwidth from the critical
    path, with descriptors posted from the idle sync queue.
  - Phase order interleaves the two samples: qkv0, S0 (s1's GroupNorm
    rides inside), qkv1, S1a, attn-tail(s0,ib0), S1b, attn-tail(s0,ib1),
    attn-tail(s1,*). Each phase's Z row-sum + 1/Z = exp(-ln Z) chain and
    its partition broadcast (zb) are hoisted at least one phase ahead of
    the O evacuations that consume them, so the PE and DVE never wait on
    the ACT ln/exp chain. The final i-block's output DMA is split per
    channel tile across three queues to shorten the drain.
"""

import math
import sys

import numpy as np

try:
    import concourse.bass as bass
except ImportError:  # pragma: no cover - grading container path setup
    sys.path.insert(0, "/opt/trn_rl_repo")
    import concourse.bass as bass

import bass_rust
import ml_dtypes
import concourse.tile as tile
from concourse import mybir
from concourse.bass_utils import run_bass_kernel_spmd

F32 = mybir.dt.float32
BF16 = mybir.dt.bfloat16
FP8 = mybir.dt.float8e4
DR = mybir.MatmulPerfMode.DoubleRow
AF = mybir.ActivationFunctionType
OP = mybir.AluOpType

NCORES = 8
B = 16
S = B // NCORES  # samples per core
C = 512
N = 1024  # H*W
G = 8  # groups
EPS = 1e-5
CT = C // 128  # channel p-tiles (4)
NT = N // 128  # spatial p-tiles (8)
IBS = 512  # i-block size
IB = N // IBS  # i blocks (2)
INV_SQRT_C = 1.0 / math.sqrt(C)

# Settable by test harness for profiling; not used by the grader.
TRACE = False
LAST_RESULT = None


MAX_WAITS = 1


def _split_excess_waits(nc, max_waits=MAX_WAITS):
    """Workaround for a walrus codegen limit: an instruction may carry at
    most `max_waits` semaphore waits ("Too many sync wait commands").
    Move the excess onto a chain of NOPs on the same engine right before
    the instruction - sequentially blocking waits on one engine queue are
    semantically identical to one multi-wait instruction."""
    counter = 0
    for f in nc.m.functions:
        for blk in f.blocks:
            il = blk.instructions
            if not any(
                i.sync_info is not None and len(i.sync_info.on_wait) > max_waits
                for i in il
            ):
                continue
            old = list(il)
            il.clear()
            for ins in old:
                si = ins.sync_info
                waits = list(si.on_wait) if si is not None else []
                if len(waits) > max_waits:
                    excess, keep = waits[:-max_waits], waits[-max_waits:]
                    for i0 in range(0, len(excess), max_waits):
                        counter += 1
                        nop = mybir.InstNoOp(
                            name=f"waitsplit-{counter}",
                            engine=ins.engine,
                            ins=[],
                            outs=[],
                            sync_info=bass_rust.SyncInfo(
                                on_wait=excess[i0 : i0 + max_waits], on_update=[]
                            ),
                        )
                        nc.register_instruction(nop, overwrite=True)
                        blk.add_instruction(nop)
                    ins.sync_info = bass_rust.SyncInfo(
                        on_wait=keep, on_update=list(si.on_update)
                    )
                blk.add_instruction(ins)
    return counter


def _build():
    from contextlib import ExitStack

    nc = bass.Bass()
    xd = nc.declare_dram_parameter("x", [S, C, N], BF16, isOutput=False)
    wald = nc.declare_dram_parameter("wall", [128, 4 * CT, C], FP8, isOutput=False)
    bald = nc.declare_dram_parameter("ball", [128, 2 * CT], F32, isOutput=False)
    cad = nc.declare_dram_parameter("ca", [128, 2], BF16, isOutput=False)
    cbd = nc.declare_dram_parameter("cb", [2, 128], BF16, isOutput=False)
    outd = nc.declare_dram_parameter("out", [S, C, N], BF16, isOutput=True)

    x_ap = xd[:].rearrange("s (t p) n -> s t p n", p=128)
    out_ap = outd[:].rearrange("s (t p) n -> s p t n", p=128)

    with tile.TileContext(nc) as tc, ExitStack() as ctx:
        singles = ctx.enter_context(tc.tile_pool(name="singles", bufs=1))
        xp = ctx.enter_context(tc.tile_pool(name="xp", bufs=S))
        xnp = ctx.enter_context(tc.tile_pool(name="xnp", bufs=S))
        qp = ctx.enter_context(tc.tile_pool(name="qp", bufs=2))
        kp = ctx.enter_context(tc.tile_pool(name="kp", bufs=2))
        vp = ctx.enter_context(tc.tile_pool(name="vp", bufs=2))
        ep = ctx.enter_context(tc.tile_pool(name="ep", bufs=16))
        op_ = ctx.enter_context(tc.tile_pool(name="op", bufs=3))
        rzp = ctx.enter_context(tc.tile_pool(name="rzp", bufs=6))
        obp = ctx.enter_context(tc.tile_pool(name="obp", bufs=4))
        smp = ctx.enter_context(tc.tile_pool(name="smp", bufs=4))
        ps = ctx.enter_context(tc.tile_pool(name="ps", bufs=1, space="PSUM"))

        # ---- wave-A input DMAs: sample 0's x, wq, one-hot constants ----
        wall = singles.tile([128, 4 * CT, C], FP8)
        ball = singles.tile([128, 2 * CT], F32)
        ca = singles.tile([128, 2], BF16)
        cb = singles.tile([2, 128], BF16)
        xs = []
        for s in range(S):
            x_sb = xp.tile([128, CT, N], BF16, tag="x", name=f"x{s}")
            xs.append(x_sb)
        # dummy activation at the head of the ACT queue so the ~1.3us
        # activation-table load runs during the DMA ramp, not after it
        dummy = singles.tile([1, 1], F32)
        nc.scalar.memzero(dummy[:])
        nc.scalar.activation(out=dummy[:], in_=dummy[:], func=AF.Exp)
        # half-tile descriptors so stats can start on the first 512 columns
        # while the rest streams in; three queues post in parallel so tiles
        # t0/t1/t2 complete together, then t3
        for t, eng in ((0, nc.sync), (1, nc.gpsimd), (2, nc.scalar),
                       (3, nc.scalar)):
            eng.dma_start(out=xs[0][:, t, 0:512], in_=x_ap[0, t][:, 0:512])
            eng.dma_start(out=xs[0][:, t, 512:1024], in_=x_ap[0, t][:, 512:1024])
        nc.sync.dma_start(out=wall[:, 0:CT, :], in_=wald[:, 0:CT, :])
        nc.gpsimd.dma_start(out=ca[:], in_=cad[:])
        nc.gpsimd.dma_start(out=cb[:], in_=cbd[:])
        eps_sb = singles.tile([2, 1], F32)
        nc.vector.memset(eps_sb[:], EPS)
        ones8 = singles.tile([128, 2, 16], FP8)
        nc.vector.memset(ones8[:], 1.0)
        ones_row = singles.tile([1, 128], BF16)
        nc.vector.memset(ones_row[:], 1.0)


        # ---- wave-B DMAs, gated behind wave A's last x tile: tiny WAW
        # dep-copies write a few bytes into each wave-B destination, so
        # every wave-B DMA write-after-write depends on x tile 3. Emitted
        # (priority-wise) after sample 0's stats so the gpsimd staging
        # smalls are not delayed; correctness rides on the data deps. ----
        def emit_waveb():
            gate = xs[0][:, 3, 512:513]
            for dst in (
                xs[1][:, 0, 0:1],
                xs[1][:, 1, 0:1],
                xs[1][:, 2, 0:1],
                xs[1][:, 3, 0:1],
            ):
                nc.gpsimd.tensor_copy(out=dst, in_=gate)
            gate8 = singles.tile([128, 1], FP8, name="gate8")
            nc.gpsimd.tensor_copy(out=gate8[:], in_=gate)
            for wi in range(1, 4):
                nc.gpsimd.tensor_copy(out=wall[:, wi * CT, 0:1], in_=gate8[:])
            gatef = singles.tile([128, 1], F32, name="gatef")
            nc.gpsimd.tensor_copy(out=gatef[:], in_=gate)
            nc.gpsimd.tensor_copy(out=ball[:, 0:1], in_=gatef[:])
            # wave-B descriptors ride the sync queue (idle after wave A) so
            # the gpsimd queue stays free for the GroupNorm staging smalls
            for t in range(CT):
                nc.sync.dma_start(out=xs[1][:, t, :], in_=x_ap[1, t])
            for wi in range(1, 4):
                nc.sync.dma_start(
                    out=wall[:, wi * CT : (wi + 1) * CT, :],
                    in_=wald[:, wi * CT : (wi + 1) * CT, :],
                )
            nc.sync.dma_start(out=ball[:], in_=bald[:])

        # weight planes for DoubleRow: [p, wi, g, q, o]; channel = 256g+128q+p
        w8 = wall.rearrange("p (w g q) f -> p w g q f", g=2, q=2)
        bq, pb = (ball[:, i * CT : (i + 1) * CT] for i in range(2))
        ghot, hhot = ca[:, 0:2], cb[:, :]

        xns = [None] * S
        sa_hs = [None] * S
        qkvs = [None] * S
        es_all = [[None] * IB for _ in range(S)]
        rzs = [[None] * IB for _ in range(S)]

        def emit_gn_alloc(s):
            xns[s] = [
                xnp.tile([128, 2, N], FP8, tag="xn", bufs=2 * S, name=f"xn{s}{g}")
                for g in range(2)
            ]
            sa_hs[s] = [
                smp.tile([128, 4], BF16, tag="sa", name=f"sa{s}{h}")
                for h in range(2)
            ]

        scr = singles.tile([128, N], BF16, name="scr")

        def emit_gn_stats_act(s, tiles):
            """ACT stats path (activation + free-dim accumulate); runs in
            parallel with the DVE bn_stats path to cut startup latency."""
            x_sb = xs[s]
            for t in tiles:
                sa = sa_hs[s][t // 2]
                c0 = 2 * (t % 2)
                sum3 = smp.tile([128, 1], F32, tag="sum3")
                nc.scalar.activation(
                    out=scr[:], in_=x_sb[:, t, :], func=AF.Identity,
                    accum_out=sum3[:],
                )
                sq3 = smp.tile([128, 1], F32, tag="sq3")
                nc.scalar.activation(
                    out=scr[:], in_=x_sb[:, t, :], func=AF.Square,
                    accum_out=sq3[:],
                )
                nc.scalar.activation(
                    out=sa[:, c0 : c0 + 1], in_=sum3[:], func=AF.Identity,
                    scale=1.0 / N,
                )
                nc.scalar.activation(
                    out=sa[:, c0 + 1 : c0 + 2], in_=sq3[:], func=AF.Identity,
                    scale=1.0 / N,
                )

        def emit_gn_stats(s, tiles):
            """DVE bn_stats; the [mean, E[x^2]] staging smalls ride on the
            otherwise-idle GPSIMD engine (fast at [128,1] sizes) so the DVE
            serial chain is just bn_stats + bn_aggr."""
            x_sb = xs[s]
            for t in tiles:
                sa = sa_hs[s][t // 2]
                c0 = 2 * (t % 2)
                st6 = smp.tile([128, 2, 6], F32, tag="st6")
                nc.vector.bn_stats(out=st6[:, 0, :], in_=x_sb[:, t, 0:512])
                nc.vector.bn_stats(out=st6[:, 1, :], in_=x_sb[:, t, 512:1024])
                mv = smp.tile([128, 2], F32, tag="mv")
                nc.vector.bn_aggr(out=mv[:], in_=st6[:])
                nc.gpsimd.tensor_copy(out=sa[:, c0 : c0 + 1], in_=mv[:, 0:1])
                msq = smp.tile([128, 1], F32, tag="msq")
                nc.gpsimd.tensor_mul(msq[:], mv[:, 0:1], mv[:, 0:1])
                nc.gpsimd.tensor_tensor(
                    out=sa[:, c0 + 1 : c0 + 2],
                    in0=mv[:, 1:2],
                    in1=msq[:],
                    op=OP.add,
                )

        bcss = [[None, None] for _ in range(S)]

        def emit_gn_chain(s, h0):
            """group reduce -> rstd -> broadcast for tile pair h0."""
            gs_ps = ps.tile([2, 4], F32, tag="zbp", bufs=1, name=f"gs{s}{h0}")
            nc.tensor.matmul(
                gs_ps[:], lhsT=ghot, rhs=sa_hs[s][h0][:], start=True, stop=True
            )
            gs3 = gs_ps.rearrange("h (t s) -> h t s", s=2)
            sq = smp.tile([2, 2], F32, tag="sq")
            nc.scalar.activation(out=sq[:], in_=gs3[:, :, 0], func=AF.Square)
            var = smp.tile([2, 2], F32, tag="var")
            nc.vector.tensor_tensor(
                out=var[:], in0=gs3[:, :, 1], in1=sq[:], op=OP.subtract
            )
            lnv = smp.tile([2, 2], F32, tag="lnv")
            nc.scalar.activation(
                out=lnv[:], in_=var[:], func=AF.Ln, bias=eps_sb[:], scale=1.0
            )
            # vals: (rstd, -mean*rstd); apply is x*rstd + (-mean*rstd),
            # expressible on DVE (tensor_scalar) and ACT (scale/bias)
            vals = smp.tile([2, 4], BF16, tag="vals")
            vals3 = vals.rearrange("h (t s) -> h t s", s=2)
            nc.scalar.activation(
                out=vals3[:, :, 0], in_=lnv[:], func=AF.Exp, scale=-0.5
            )
            nc.vector.scalar_tensor_tensor(
                out=vals3[:, :, 1], in0=gs3[:, :, 0], scalar=-1.0,
                in1=vals3[:, :, 0], op0=OP.mult, op1=OP.mult,
            )
            bc = ps.tile([128, 4], F32, tag="zbp", bufs=1, name=f"bc{s}{h0}")
            nc.tensor.matmul(bc[:], lhsT=hhot, rhs=vals[:], start=True, stop=True)
            bcs = smp.tile([128, 4], F32, tag="bcs")
            nc.vector.tensor_copy(out=bcs[:], in_=bc[:])
            bcss[s][h0] = bcs

        def emit_gn_apply(s, h0, engines=("v", "v")):
            bcs = bcss[s][h0]
            for tt in range(2):
                t = 2 * h0 + tt
                if engines[tt] == "v":
                    nc.vector.tensor_scalar(
                        out=xns[s][h0][:, tt, :],
                        in0=xs[s][:, t, :],
                        scalar1=bcs[:, 2 * tt : 2 * tt + 1],
                        scalar2=bcs[:, 2 * tt + 1 : 2 * tt + 2],
                        op0=OP.mult,
                        op1=OP.add,
                    )
                else:  # ACT: out = Identity(x*rstd + (-mean*rstd))
                    nc.scalar.activation(
                        out=xns[s][h0][:, tt, :],
                        in_=xs[s][:, t, :],
                        func=AF.Identity,
                        scale=bcs[:, 2 * tt : 2 * tt + 1],
                        bias=bcs[:, 2 * tt + 1 : 2 * tt + 2],
                    )

        def emit_qkv(s):
            xn_g = xns[s]
            q_sb = qp.tile([128, 2, 2, N], FP8, tag="q")
            k_sb = kp.tile([128, 2, 2, N], FP8, tag="k")
            v_sb = vp.tile([128, NT // 2, 2, C], FP8, tag="v")
            qkvs[s] = (q_sb, k_sb, v_sb)
            qv = q_sb.rearrange("p g q n -> p (g q) n")
            kv = k_sb.rearrange("p g q n -> p (g q) n")
            # per-tile (half-used pair slot) PSUM so each evacuation
            # depends only on its own matmuls, not a whole merged pair
            for wi, dst in ((0, qv), (1, kv)):
                for ot in range(CT):
                    for ib in range(IB):
                        psm = ps.tile([128, 2, IBS], F32, tag="mmp", bufs=3)
                        for g in range(2):
                            nc.tensor.matmul(
                                psm[:, 0, :],
                                lhsT=w8[:, wi, g, :, ot * 128 : (ot + 1) * 128],
                                rhs=xn_g[g][:, :, ib * IBS : (ib + 1) * IBS],
                                start=(g == 0),
                                stop=(g == 1),
                                perf_mode=DR,
                            )
                        if wi == 0:
                            # Q bias folded into the DVE evacuation
                            nc.vector.tensor_scalar(
                                out=dst[:, ot, ib * IBS : (ib + 1) * IBS],
                                in0=psm[:, 0, :],
                                scalar1=bq[:, ot : ot + 1],
                                scalar2=None,
                                op0=OP.add,
                            )
                        else:
                            # K bias dropped (cancels in softmax): plain cast
                            nc.vector.tensor_copy(
                                out=dst[:, ot, ib * IBS : (ib + 1) * IBS],
                                in_=psm[:, 0, :],
                            )
            for nt in range(NT):
                psm = ps.tile([128, 2, IBS], F32, tag="mmp", bufs=3)
                for g in range(2):
                    nc.tensor.matmul(
                        psm[:, 0, :],
                        lhsT=xn_g[g][:, :, nt * 128 : (nt + 1) * 128],
                        rhs=w8[:, 2, g, :, :],
                        start=(g == 0),
                        stop=(g == 1),
                        perf_mode=DR,
                    )
                # V evacuations alternate ACT/DVE: ACT alone drains V at
                # half the PE's rate and the backlog stalls the S phase's
                # first PSUM-slot allocations
                if nt % 2 == 0:
                    nc.scalar.activation(
                        out=v_sb[:, nt // 2, nt % 2, :], in_=psm[:, 0, :],
                        func=AF.Identity,
                    )
                else:
                    nc.vector.tensor_copy(
                        out=v_sb[:, nt // 2, nt % 2, :], in_=psm[:, 0, :]
                    )

        def emit_S(s, ibs):
            """S^T matmuls + one wide exp per pair of j-tiles."""
            q_sb, k_sb, _ = qkvs[s]
            for ib in ibs:
                isl = slice(ib * IBS, (ib + 1) * IBS)
                es = []
                es_all[s][ib] = es
                for jp in range(NT // 2):
                    psm = ps.tile([128, 2, IBS], F32, tag="mmp", bufs=3)
                    for i in range(2):
                        jt = 2 * jp + i
                        for g in range(2):
                            nc.tensor.matmul(
                                psm[:, i, :],
                                lhsT=k_sb[:, g, :, jt * 128 : (jt + 1) * 128],
                                rhs=q_sb[:, g, :, isl],
                                start=(g == 0),
                                stop=(g == 1),
                                perf_mode=DR,
                            )
                    e = ep.tile([128, 2, IBS], FP8, tag="e")
                    es.append(e)
                    nc.scalar.activation(
                        out=e[:], in_=psm[:], func=AF.Exp, scale=INV_SQRT_C
                    )

        def emit_Z(s, ib):
            """Z row-sums + the ACT ln/exp chain for 1/Z; emitted one phase
            early so the zb broadcast never waits on ACT."""
            es = es_all[s][ib]
            zps = ps.tile([1, IBS], F32, tag="zrow", bufs=1, name=f"z{s}{ib}")
            for jg in range(NT // 2):
                nc.tensor.matmul(
                    zps[:],
                    lhsT=ones8[:, :, 0:1],
                    rhs=es[jg][:],
                    start=(jg == 0),
                    stop=(jg == NT // 2 - 1),
                    perf_mode=DR,
                )
            lnz = rzp.tile([1, IBS], F32, tag="lnz")
            nc.scalar.activation(out=lnz[:], in_=zps[:], func=AF.Ln)
            rz = rzp.tile([1, IBS], BF16, tag="rz")
            nc.scalar.activation(out=rz[:], in_=lnz[:], func=AF.Exp, scale=-1.0)
            rzs[s][ib] = rz

        zbs = [[None] * IB for _ in range(S)]

        def emit_zb(s, ib):
            """1/Z broadcast across partitions into SBUF, hoisted well ahead
            of the O evacuations that read it."""
            zb_ps = ps.tile([128, IBS], F32, tag="zbp", bufs=1,
                            name=f"zbp{s}{ib}")
            nc.tensor.matmul(
                zb_ps[:], lhsT=ones_row, rhs=rzs[s][ib][:], start=True,
                stop=True,
            )
            zb = obp.tile([128, IBS], F32, tag="zbs", bufs=4, name=f"zb{s}{ib}")
            nc.vector.tensor_copy(out=zb[:], in_=zb_ps[:])
            zbs[s][ib] = zb

        def emit_attn2(s, ib):
            """O = V E^T, proj, residual evac, output DMA."""
            x_sb = xs[s]
            _, _, v_sb = qkvs[s]
            isl = slice(ib * IBS, (ib + 1) * IBS)
            es = es_all[s][ib]
            o_sb = op_.tile([128, 2, 2, IBS], FP8, tag="o")
            ov = o_sb.rearrange("p g q n -> p (g q) n")
            zb = zbs[s][ib]
            # one (pair-shaped, half-used) PSUM tile per ct so each
            # evacuation depends only on its own 4 matmuls - group tracking
            # is tile-granular, so sharing a tile across two ct's made the
            # first evac wait for both halves and stalled the PE each phase
            for ct in range(CT):
                psm = ps.tile([128, 2, IBS], F32, tag="mmp", bufs=3)
                for jg in range(NT // 2):
                    nc.tensor.matmul(
                        psm[:, 0, :],
                        lhsT=v_sb[:, jg, :, ct * 128 : (ct + 1) * 128],
                        rhs=es[jg][:],
                        start=(jg == 0),
                        stop=(jg == NT // 2 - 1),
                        perf_mode=DR,
                    )
                nc.vector.tensor_tensor(
                    out=ov[:, ct, :], in0=psm[:, 0, :], in1=zb[:], op=OP.mult
                )
            ob4 = obp.tile([128, CT, IBS], BF16, tag="ob", bufs=3)
            for ot in range(CT):
                psm = ps.tile([128, 2, IBS], F32, tag="mmp", bufs=3)
                for g in range(2):
                    nc.tensor.matmul(
                        psm[:, 0, :],
                        lhsT=w8[:, 3, g, :, ot * 128 : (ot + 1) * 128],
                        rhs=o_sb[:, g, :, :],
                        start=(g == 0),
                        stop=(g == 1),
                        perf_mode=DR,
                    )
                # out = (psum + pb') + x in one pass
                nc.vector.scalar_tensor_tensor(
                    out=ob4[:, ot, :],
                    in0=psm[:, 0, :],
                    scalar=pb[:, ot : ot + 1],
                    in1=x_sb[:, ot, isl],
                    op0=OP.add,
                    op1=OP.add,
                )
                if s == S - 1 and ib == IB - 1:
                    # tail: one DMA per ot, spread across queues so the
                    # descriptor issue doesn't serialize the drain
                    eng = (nc.sync, nc.scalar, nc.gpsimd, nc.sync)[ot]
                    eng.dma_start(
                        out=out_ap[s][:, ot : ot + 1, isl],
                        in_=ob4[:, ot : ot + 1, :],
                    )
            if not (s == S - 1 and ib == IB - 1):
                nc.sync.dma_start(out=out_ap[s][:, :, isl], in_=ob4[:])

        # ---- software-pipelined emission across the two samples ----
        emit_gn_alloc(0)
        emit_gn_stats_act(0, [0])   # t0 on ACT, in parallel with DVE
        emit_gn_stats(0, [1, 2])
        with tc.high_priority():
            emit_gn_chain(0, 0)
            emit_gn_apply(0, 0, ("v", "s"))  # t1's apply rides on ACT
        emit_gn_stats(0, [3])
        emit_waveb()
        with tc.high_priority():
            emit_gn_chain(0, 1)
            emit_gn_apply(0, 1, ("v", "s"))
        emit_qkv(0)
        emit_S(0, [0])          # s0 S phase, first i-block
        emit_gn_alloc(1)
        emit_gn_stats(1, [0, 1, 2, 3])  # s1 stats fill idle DVE here
        # high priority: slot the s1 chain/applies in as soon as their deps
        # are ready (the scheduler otherwise defers them until right before
        # qkv1 and the cross-engine ping-pong stalls the PE there)
        with tc.high_priority():
            emit_gn_chain(1, 0)
            emit_gn_chain(1, 1)
        emit_S(0, [1])          # s0 S phase, second i-block
        with tc.high_priority():
            emit_gn_apply(1, 0)
            emit_gn_apply(1, 1)
        emit_qkv(1)             # fills the PE while s0's exps drain
        emit_Z(0, 0)
        emit_S(1, [0])          # s1's first exps drain under attn2(0,0)
        emit_zb(0, 0)
        emit_attn2(0, 0)
        emit_Z(0, 1)
        emit_S(1, [1])
        emit_Z(1, 0)            # s1 Z heads + all remaining zb broadcasts
        emit_zb(0, 1)           # early: the tail phases then have no ACT
        emit_Z(1, 1)            # dependencies at all
        emit_zb(1, 0)
        emit_attn2(0, 1)
        emit_zb(1, 1)
        emit_attn2(1, 0)
        emit_attn2(1, 1)

    _split_excess_waits(nc)
    return nc


_NC = None


def kernel(x, norm_w, norm_b, qkv_w, qkv_b, proj_w, proj_b):
    global _NC, LAST_RESULT
    x = np.asarray(x, dtype=np.float32)
    norm_w = np.asarray(norm_w, dtype=np.float32)
    norm_b = np.asarray(norm_b, dtype=np.float32)
    qkv_w = np.asarray(qkv_w, dtype=np.float32)
    qkv_b = np.asarray(qkv_b, dtype=np.float32)
    proj_w = np.asarray(proj_w, dtype=np.float32)
    proj_b = np.asarray(proj_b, dtype=np.float32)

    # fold GroupNorm affine into qkv
    wq_full = qkv_w * norm_w[None, :]
    bq_full = qkv_b + qkv_w @ norm_b
    wq_, wk_, wv_ = wq_full[0:C], wq_full[C : 2 * C], wq_full[2 * C : 3 * C]
    bq_, bv_ = bq_full[0:C], bq_full[2 * C : 3 * C]
    pb_ = proj_w @ bv_ + proj_b

    def wtile(w):  # [o, c] -> DoubleRow lhsT planes [128, 2(g), 2(q), o]
        return w.T.reshape(2, 2, 128, C).transpose(2, 0, 1, 3)

    def btile(b):  # [C] -> [128, ct]
        return b.reshape(CT, 128).T

    wall = np.ascontiguousarray(
        np.stack(
            [wtile(wq_), wtile(wk_), wtile(wv_), wtile(proj_w)], axis=1
        ).reshape(128, 16, C).astype(ml_dtypes.float8_e4m3)
    )
    ball = np.ascontiguousarray(
        np.concatenate([btile(bq_), btile(pb_)], axis=1).astype(np.float32)
    )
    cl = np.arange(128)
    ghot = np.zeros((128, 2), np.float32)
    ghot[cl, cl // 64] = 1.0 / 64.0
    hhot = np.zeros((2, 128), np.float32)
    hhot[cl // 64, cl] = 1.0

    common = {
        "wall": wall,
        "ball": ball,
        "ca": ghot.astype(ml_dtypes.bfloat16),
        "cb": hhot.astype(ml_dtypes.bfloat16),
    }
    xr = np.ascontiguousarray(
        x.reshape(NCORES, S, C, N).astype(ml_dtypes.bfloat16)
    )
    in_maps = [dict(common, x=xr[i]) for i in range(NCORES)]

    if _NC is None:
        _NC = _build()
    res = run_bass_kernel_spmd(
        _NC, in_maps, core_ids=list(range(NCORES)), trace=TRACE
    )
    LAST_RESULT = res
    out = np.stack([res.results[i]["out"] for i in range(NCORES)])
    return np.ascontiguousarray(
        out.reshape(B, C, 32, 32).astype(np.float32)
    )


# revision 61
# speedup vs baseline: 1.1826x; 1.0137x over previous
"""AttentionBlock (GroupNorm + 1x1-conv QKV + full NxN attention + proj +
residual) on 8 Trainium2 NeuronCores, data-parallel over the batch dim.

Per core: 2 samples of x[16, 512, 32, 32]. Matmul operands are fp8e4m3
with DoubleRow perf mode (2 contraction planes per instruction - the
128x128 PE array virtualizes to 128x256). PSUM accumulation stays fp32;
the residual path is bf16 (x is converted to bf16 on the host, halving
input DMA, and the output is written bf16 and upcast on the host).

Algebraic folds done on the host (exact):
  - GroupNorm affine (norm_w, norm_b) folded into qkv_w / qkv_b.
  - K bias dropped entirely: S_ji += bk.Q_i is constant along the
    softmax axis j, so it cancels between numerator and denominator.
  - V bias and proj bias folded into one per-channel bias
    pb' = proj_w @ bv + proj_b  (softmax rows sum to 1).
  - softmax denominator folded through the projection:
    out = (x + pb') + (proj_w @ (V @ E^T)) * (1/Z).

Performance structure:
  - All matmul PSUM goes through two-bank pair tiles [128,2,512] so
    evacuations are one wide op per two matmul tiles (halves ACT/DVE
    per-instruction overhead and PSUM-bank grab events); the exp of
    S^T is a single [128,2,512] ACT op per pair.
  - GroupNorm stats are split ACT (tile 0 via activation+accum) / DVE
    (tiles 1-3 via bn_stats, staging smalls on the idle GPSIMD) and the
    normalize applies are split DVE/ACT (vals carry -mean*rstd so both
    engines express x*rstd - mean*rstd), cutting startup latency; x
    arrives as half-tile descriptors over three DMA queues.
  - Input DMA is two waves: wave A (sample 0's x, the Q weights, the
    one-hot constants) goes immediately; wave B (remaining weights,
    biases, sample 1's x) is gated behind wave A's last x tile by tiny
    WAW dep-copies so it cannot steal HBM bandwidth from the critical
    path, with descriptors posted from the idle sync queue.
  - Phase order interleaves the two samples: qkv0, S0 (s1's GroupNorm
    rides inside), qkv1, S1a, attn-tail(s0,ib0), S1b, attn-tail(s0,ib1),
    attn-tail(s1,*). Each phase's Z row-sum + 1/Z = exp(-ln Z) chain and
    its partition broadcast (zb) are hoisted at least one phase ahead of
    the O evacuations that consume them, so the PE and DVE never wait on
    the ACT ln/exp chain. The final i-block's output DMA is split per
    channel tile across three queues to shorten the drain.
"""

import math
import sys

import numpy as np

try:
    import concourse.bass as bass
except ImportError:  # pragma: no cover - grading container path setup
    sys.path.insert(0, "/opt/trn_rl_repo")
    import concourse.bass as bass

import bass_rust
import ml_dtypes
import concourse.tile as tile
from concourse import mybir
from concourse.bass_utils import run_bass_kernel_spmd

F32 = mybir.dt.float32
BF16 = mybir.dt.bfloat16
FP8 = mybir.dt.float8e4
DR = mybir.MatmulPerfMode.DoubleRow
AF = mybir.ActivationFunctionType
OP = mybir.AluOpType

NCORES = 8
B = 16
S = B // NCORES  # samples per core
C = 512
N = 1024  # H*W
G = 8  # groups
EPS = 1e-5
CT = C // 128  # channel p-tiles (4)
NT = N // 128  # spatial p-tiles (8)
IBS = 512  # i-block size
IB = N // IBS  # i blocks (2)
INV_SQRT_C = 1.0 / math.sqrt(C)

# Settable by test harness for profiling; not used by the grader.
TRACE = False
LAST_RESULT = None


MAX_WAITS = 1


def _split_excess_waits(nc, max_waits=MAX_WAITS):
    """Workaround for a walrus codegen limit: an instruction may carry at
    most `max_waits` semaphore waits ("Too many sync wait commands").
    Move the excess onto a chain of NOPs on the same engine right before
    the instruction - sequentially blocking waits on one engine queue are
    semantically identical to one multi-wait instruction."""
    counter = 0
    for f in nc.m.functions:
        for blk in f.blocks:
            il = blk.instructions
            if not any(
                i.sync_info is not None and len(i.sync_info.on_wait) > max_waits
                for i in il
            ):
                continue
            old = list(il)
            il.clear()
            for ins in old:
                si = ins.sync_info
                waits = list(si.on_wait) if si is not None else []
                if len(waits) > max_waits:
                    excess, keep = waits[:-max_waits], waits[-max_waits:]
                    for i0 in range(0, len(excess), max_waits):
                        counter += 1
                        nop = mybir.InstNoOp(
                            name=f"waitsplit-{counter}",
                            engine=ins.engine,
                            ins=[],
                            outs=[],
                            sync_info=bass_rust.SyncInfo(
                                on_wait=excess[i0 : i0 + max_waits], on_update=[]
                            ),
                        )
                        nc.register_instruction(nop, overwrite=True)
                        blk.add_instruction(nop)
                    ins.sync_info = bass_rust.SyncInfo(
                        on_wait=keep, on_update=list(si.on_update)
                    )
                blk.add_instruction(ins)
    return counter


def _build():
    from contextlib import ExitStack

    nc = bass.Bass()
    xd = nc.declare_dram_parameter("x", [S, C, N], BF16, isOutput=False)
    wald = nc.declare_dram_parameter("wall", [128, 4 * CT, C], FP8, isOutput=False)
    bald = nc.declare_dram_parameter("ball", [128, 2 * CT], F32, isOutput=False)
    cad = nc.declare_dram_parameter("ca", [128, 2], BF16, isOutput=False)
    cbd = nc.declare_dram_parameter("cb", [2, 128], BF16, isOutput=False)
    outd = nc.declare_dram_parameter("out", [S, C, N], BF16, isOutput=True)

    x_ap = xd[:].rearrange("s (t p) n -> s t p n", p=128)
    out_ap = outd[:].rearrange("s (t p) n -> s p t n", p=128)

    with tile.TileContext(nc) as tc, ExitStack() as ctx:
        singles = ctx.enter_context(tc.tile_pool(name="singles", bufs=1))
        xp = ctx.enter_context(tc.tile_pool(name="xp", bufs=S))
        xnp = ctx.enter_context(tc.tile_pool(name="xnp", bufs=S))
        qp = ctx.enter_context(tc.tile_pool(name="qp", bufs=2))
        kp = ctx.enter_context(tc.tile_pool(name="kp", bufs=2))
        vp = ctx.enter_context(tc.tile_pool(name="vp", bufs=2))
        ep = ctx.enter_context(tc.tile_pool(name="ep", bufs=16))
        op_ = ctx.enter_context(tc.tile_pool(name="op", bufs=3))
        rzp = ctx.enter_context(tc.tile_pool(name="rzp", bufs=6))
        obp = ctx.enter_context(tc.tile_pool(name="obp", bufs=4))
        smp = ctx.enter_context(tc.tile_pool(name="smp", bufs=4))
        ps = ctx.enter_context(tc.tile_pool(name="ps", bufs=1, space="PSUM"))

        # ---- wave-A input DMAs: sample 0's x, wq, one-hot constants ----
        wall = singles.tile([128, 4 * CT, C], FP8)
        ball = singles.tile([128, 2 * CT], F32)
        ca = singles.tile([128, 2], BF16)
        cb = singles.tile([2, 128], BF16)
        xs = []
        for s in range(S):
            x_sb = xp.tile([128, CT, N], BF16, tag="x", name=f"x{s}")
            xs.append(x_sb)
        # dummy activation at the head of the ACT queue so the ~1.3us
        # activation-table load runs during the DMA ramp, not after it
        dummy = singles.tile([1, 1], F32)
        nc.scalar.memzero(dummy[:])
        nc.scalar.activation(out=dummy[:], in_=dummy[:], func=AF.Exp)
        # half-tile descriptors so stats can start on the first 512 columns
        # while the rest streams in; three queues post in parallel so tiles
        # t0/t1/t2 complete together, then t3
        for t, eng in ((0, nc.sync), (1, nc.gpsimd), (2, nc.scalar),
                       (3, nc.scalar)):
            eng.dma_start(out=xs[0][:, t, 0:512], in_=x_ap[0, t][:, 0:512])
            eng.dma_start(out=xs[0][:, t, 512:1024], in_=x_ap[0, t][:, 512:1024])
        nc.sync.dma_start(out=wall[:, 0:CT, :], in_=wald[:, 0:CT, :])
        nc.gpsimd.dma_start(out=ca[:], in_=cad[:])
        nc.gpsimd.dma_start(out=cb[:], in_=cbd[:])
        eps_sb = singles.tile([2, 1], F32)
        nc.vector.memset(eps_sb[:], EPS)
        ones8 = singles.tile([128, 2, 16], FP8)
        nc.vector.memset(ones8[:], 1.0)
        ones_row = singles.tile([1, 128], BF16)
        nc.vector.memset(ones_row[:], 1.0)


        # ---- wave-B DMAs, gated behind wave A's last x tile: tiny WAW
        # dep-copies write a few bytes into each wave-B destination, so
        # every wave-B DMA write-after-write depends on x tile 3. Emitted
        # (priority-wise) after sample 0's stats so the gpsimd staging
        # smalls are not delayed; correctness rides on the data deps. ----
        def emit_waveb():
            gate = xs[0][:, 3, 512:513]
            for dst in (
                xs[1][:, 0, 0:1],
                xs[1][:, 1, 0:1],
                xs[1][:, 2, 0:1],
                xs[1][:, 3, 0:1],
            ):
                nc.gpsimd.tensor_copy(out=dst, in_=gate)
            gate8 = singles.tile([128, 1], FP8, name="gate8")
            nc.gpsimd.tensor_copy(out=gate8[:], in_=gate)
            for wi in range(1, 4):
                nc.gpsimd.tensor_copy(out=wall[:, wi * CT, 0:1], in_=gate8[:])
            gatef = singles.tile([128, 1], F32, name="gatef")
            nc.gpsimd.tensor_copy(out=gatef[:], in_=gate)
            nc.gpsimd.tensor_copy(out=ball[:, 0:1], in_=gatef[:])
            # wave-B descriptors ride the sync queue (idle after wave A) so
            # the gpsimd queue stays free for the GroupNorm staging smalls
            for t in range(CT):
                nc.sync.dma_start(out=xs[1][:, t, :], in_=x_ap[1, t])
            for wi in range(1, 4):
                nc.sync.dma_start(
                    out=wall[:, wi * CT : (wi + 1) * CT, :],
                    in_=wald[:, wi * CT : (wi + 1) * CT, :],
                )
            nc.sync.dma_start(out=ball[:], in_=bald[:])

        # weight planes for DoubleRow: [p, wi, g, q, o]; channel = 256g+128q+p
        w8 = wall.rearrange("p (w g q) f -> p w g q f", g=2, q=2)
        bq, pb = (ball[:, i * CT : (i + 1) * CT] for i in range(2))
        ghot, hhot = ca[:, 0:2], cb[:, :]

        xns = [None] * S
        sa_hs = [None] * S
        qkvs = [None] * S
        es_all = [[None] * IB for _ in range(S)]
        rzs = [[None] * IB for _ in range(S)]

        def emit_gn_alloc(s):
            xns[s] = [
                xnp.tile([128, 2, N], FP8, tag="xn", bufs=2 * S, name=f"xn{s}{g}")
                for g in range(2)
            ]
            sa_hs[s] = [
                smp.tile([128, 4], BF16, tag="sa", name=f"sa{s}{h}")
                for h in range(2)
            ]

        scr = singles.tile([128, N], BF16, name="scr")

        def emit_gn_stats_act(s, tiles):
            """ACT stats path (activation + free-dim accumulate); runs in
            parallel with the DVE bn_stats path to cut startup latency."""
            x_sb = xs[s]
            for t in tiles:
                sa = sa_hs[s][t // 2]
                c0 = 2 * (t % 2)
                sum3 = smp.tile([128, 1], F32, tag="sum3")
                nc.scalar.activation(
                    out=scr[:], in_=x_sb[:, t, :], func=AF.Identity,
                    accum_out=sum3[:],
                )
                sq3 = smp.tile([128, 1], F32, tag="sq3")
                nc.scalar.activation(
                    out=scr[:], in_=x_sb[:, t, :], func=AF.Square,
                    accum_out=sq3[:],
                )
                nc.scalar.activation(
                    out=sa[:, c0 : c0 + 1], in_=sum3[:], func=AF.Identity,
                    scale=1.0 / N,
                )
                nc.scalar.activation(
                    out=sa[:, c0 + 1 : c0 + 2], in_=sq3[:], func=AF.Identity,
                    scale=1.0 / N,
                )

        def emit_gn_stats(s, tiles):
            """DVE bn_stats; the [mean, E[x^2]] staging smalls ride on the
            otherwise-idle GPSIMD engine (fast at [128,1] sizes) so the DVE
            serial chain is just bn_stats + bn_aggr."""
            x_sb = xs[s]
            for t in tiles:
                sa = sa_hs[s][t // 2]
                c0 = 2 * (t % 2)
                st6 = smp.tile([128, 2, 6], F32, tag="st6")
                nc.vector.bn_stats(out=st6[:, 0, :], in_=x_sb[:, t, 0:512])
                nc.vector.bn_stats(out=st6[:, 1, :], in_=x_sb[:, t, 512:1024])
                mv = smp.tile([128, 2], F32, tag="mv")
                nc.vector.bn_aggr(out=mv[:], in_=st6[:])
                nc.gpsimd.tensor_copy(out=sa[:, c0 : c0 + 1], in_=mv[:, 0:1])
                msq = smp.tile([128, 1], F32, tag="msq")
                nc.gpsimd.tensor_mul(msq[:], mv[:, 0:1], mv[:, 0:1])
                nc.gpsimd.tensor_tensor(
                    out=sa[:, c0 + 1 : c0 + 2],
                    in0=mv[:, 1:2],
                    in1=msq[:],
                    op=OP.add,
                )

        bcss = [[None, None] for _ in range(S)]

        def emit_gn_chain(s, h0):
            """group reduce -> rstd -> broadcast for tile pair h0."""
            gs_ps = ps.tile([2, 4], F32, tag="zbp", bufs=1, name=f"gs{s}{h0}")
            nc.tensor.matmul(
                gs_ps[:], lhsT=ghot, rhs=sa_hs[s][h0][:], start=True, stop=True
            )
            gs3 = gs_ps.rearrange("h (t s) -> h t s", s=2)
            sq = smp.tile([2, 2], F32, tag="sq")
            nc.scalar.activation(out=sq[:], in_=gs3[:, :, 0], func=AF.Square)
            var = smp.tile([2, 2], F32, tag="var")
            nc.vector.tensor_tensor(
                out=var[:], in0=gs3[:, :, 1], in1=sq[:], op=OP.subtract
            )
            lnv = smp.tile([2, 2], F32, tag="lnv")
            nc.scalar.activation(
                out=lnv[:], in_=var[:], func=AF.Ln, bias=eps_sb[:], scale=1.0
            )
            # vals: (rstd, -mean*rstd); apply is x*rstd + (-mean*rstd),
            # expressible on DVE (tensor_scalar) and ACT (scale/bias)
            vals = smp.tile([2, 4], BF16, tag="vals")
            vals3 = vals.rearrange("h (t s) -> h t s", s=2)
            nc.scalar.activation(
                out=vals3[:, :, 0], in_=lnv[:], func=AF.Exp, scale=-0.5
            )
            nc.vector.scalar_tensor_tensor(
                out=vals3[:, :, 1], in0=gs3[:, :, 0], scalar=-1.0,
                in1=vals3[:, :, 0], op0=OP.mult, op1=OP.mult,
            )
            bc = ps.tile([128, 4], F32, tag="zbp", bufs=1, name=f"bc{s}{h0}")
            nc.tensor.matmul(bc[:], lhsT=hhot, rhs=vals[:], start=True, stop=True)
            bcs = smp.tile([128, 4], F32, tag="bcs")
            nc.vector.tensor_copy(out=bcs[:], in_=bc[:])
            bcss[s][h0] = bcs

        def emit_gn_apply(s, h0, engines=("v", "v")):
            bcs = bcss[s][h0]
            for tt in range(2):
                t = 2 * h0 + tt
                if engines[tt] == "v":
                    nc.vector.tensor_scalar(
                        out=xns[s][h0][:, tt, :],
                        in0=xs[s][:, t, :],
                        scalar1=bcs[:, 2 * tt : 2 * tt + 1],
                        scalar2=bcs[:, 2 * tt + 1 : 2 * tt + 2],
                        op0=OP.mult,
                        op1=OP.add,
                    )
                else:  # ACT: out = Identity(x*rstd + (-mean*rstd))
                    nc.scalar.activation(
                        out=xns[s][h0][:, tt, :],
                        in_=xs[s][:, t, :],
                        func=AF.Identity,
                        scale=bcs[:, 2 * tt : 2 * tt + 1],
                        bias=bcs[:, 2 * tt + 1 : 2 * tt + 2],
                    )

        def emit_qkv(s):
            xn_g = xns[s]
            q_sb = qp.tile([128, 2, 2, N], FP8, tag="q")
            k_sb = kp.tile([128, 2, 2, N], FP8, tag="k")
            v_sb = vp.tile([128, NT // 2, 2, C], FP8, tag="v")
            qkvs[s] = (q_sb, k_sb, v_sb)
            qv = q_sb.rearrange("p g q n -> p (g q) n")
            kv = k_sb.rearrange("p g q n -> p (g q) n")
            # per-tile (half-used pair slot) PSUM so each evacuation
            # depends only on its own matmuls, not a whole merged pair
            for wi, dst in ((0, qv), (1, kv)):
                for ot in range(CT):
                    for ib in range(IB):
                        psm = ps.tile([128, 2, IBS], F32, tag="mmp", bufs=3)
                        for g in range(2):
                            nc.tensor.matmul(
                                psm[:, 0, :],
                                lhsT=w8[:, wi, g, :, ot * 128 : (ot + 1) * 128],
                                rhs=xn_g[g][:, :, ib * IBS : (ib + 1) * IBS],
                                start=(g == 0),
                                stop=(g == 1),
                                perf_mode=DR,
                            )
                        if wi == 0:
                            # Q bias folded into the DVE evacuation
                            nc.vector.tensor_scalar(
                                out=dst[:, ot, ib * IBS : (ib + 1) * IBS],
                                in0=psm[:, 0, :],
                                scalar1=bq[:, ot : ot + 1],
                                scalar2=None,
                                op0=OP.add,
                            )
                        else:
                            # K bias dropped (cancels in softmax): plain cast
                            nc.vector.tensor_copy(
                                out=dst[:, ot, ib * IBS : (ib + 1) * IBS],
                                in_=psm[:, 0, :],
                            )
            for nt in range(NT):
                psm = ps.tile([128, 2, IBS], F32, tag="mmp", bufs=3)
                for g in range(2):
                    nc.tensor.matmul(
                        psm[:, 0, :],
                        lhsT=xn_g[g][:, :, nt * 128 : (nt + 1) * 128],
                        rhs=w8[:, 2, g, :, :],
                        start=(g == 0),
                        stop=(g == 1),
                        perf_mode=DR,
                    )
                # V evacuation on ACT (DVE is the busier engine here)
                nc.scalar.activation(
                    out=v_sb[:, nt // 2, nt % 2, :], in_=psm[:, 0, :],
                    func=AF.Identity,
                )

        def emit_S(s, ibs):
            """S^T matmuls + one wide exp per pair of j-tiles."""
            q_sb, k_sb, _ = qkvs[s]
            for ib in ibs:
                isl = slice(ib * IBS, (ib + 1) * IBS)
                es = []
                es_all[s][ib] = es
                for jp in range(NT // 2):
                    psm = ps.tile([128, 2, IBS], F32, tag="mmp", bufs=3)
                    for i in range(2):
                        jt = 2 * jp + i
                        for g in range(2):
                            nc.tensor.matmul(
                                psm[:, i, :],
                                lhsT=k_sb[:, g, :, jt * 128 : (jt + 1) * 128],
                                rhs=q_sb[:, g, :, isl],
                                start=(g == 0),
                                stop=(g == 1),
                                perf_mode=DR,
                            )
                    e = ep.tile([128, 2, IBS], FP8, tag="e")
                    es.append(e)
                    nc.scalar.activation(
                        out=e[:], in_=psm[:], func=AF.Exp, scale=INV_SQRT_C
                    )

        def emit_Z(s, ib):
            """Z row-sums + the ACT ln/exp chain for 1/Z; emitted one phase
            early so the zb broadcast never waits on ACT."""
            es = es_all[s][ib]
            zps = ps.tile([1, IBS], F32, tag="zrow", bufs=1, name=f"z{s}{ib}")
            for jg in range(NT // 2):
                nc.tensor.matmul(
                    zps[:],
                    lhsT=ones8[:, :, 0:1],
                    rhs=es[jg][:],
                    start=(jg == 0),
                    stop=(jg == NT // 2 - 1),
                    perf_mode=DR,
                )
            lnz = rzp.tile([1, IBS], F32, tag="lnz")
            nc.scalar.activation(out=lnz[:], in_=zps[:], func=AF.Ln)
            rz = rzp.tile([1, IBS], BF16, tag="rz")
            nc.scalar.activation(out=rz[:], in_=lnz[:], func=AF.Exp, scale=-1.0)
            rzs[s][ib] = rz

        zbs = [[None] * IB for _ in range(S)]

        def emit_zb(s, ib):
            """1/Z broadcast across partitions into SBUF, hoisted well ahead
            of the O evacuations that read it."""
            zb_ps = ps.tile([128, IBS], F32, tag="zbp", bufs=1,
                            name=f"zbp{s}{ib}")
            nc.tensor.matmul(
                zb_ps[:], lhsT=ones_row, rhs=rzs[s][ib][:], start=True,
                stop=True,
            )
            zb = obp.tile([128, IBS], F32, tag="zbs", bufs=4, name=f"zb{s}{ib}")
            nc.vector.tensor_copy(out=zb[:], in_=zb_ps[:])
            zbs[s][ib] = zb

        def emit_attn2(s, ib):
            """O = V E^T, proj, residual evac, output DMA."""
            x_sb = xs[s]
            _, _, v_sb = qkvs[s]
            isl = slice(ib * IBS, (ib + 1) * IBS)
            es = es_all[s][ib]
            o_sb = op_.tile([128, 2, 2, IBS], FP8, tag="o")
            ov = o_sb.rearrange("p g q n -> p (g q) n")
            zb = zbs[s][ib]
            # one (pair-shaped, half-used) PSUM tile per ct so each
            # evacuation depends only on its own 4 matmuls - group tracking
            # is tile-granular, so sharing a tile across two ct's made the
            # first evac wait for both halves and stalled the PE each phase
            for ct in range(CT):
                psm = ps.tile([128, 2, IBS], F32, tag="mmp", bufs=3)
                for jg in range(NT // 2):
                    nc.tensor.matmul(
                        psm[:, 0, :],
                        lhsT=v_sb[:, jg, :, ct * 128 : (ct + 1) * 128],
                        rhs=es[jg][:],
                        start=(jg == 0),
                        stop=(jg == NT // 2 - 1),
                        perf_mode=DR,
                    )
                nc.vector.tensor_tensor(
                    out=ov[:, ct, :], in0=psm[:, 0, :], in1=zb[:], op=OP.mult
                )
            ob4 = obp.tile([128, CT, IBS], BF16, tag="ob", bufs=3)
            for ot in range(CT):
                psm = ps.tile([128, 2, IBS], F32, tag="mmp", bufs=3)
                for g in range(2):
                    nc.tensor.matmul(
                        psm[:, 0, :],
                        lhsT=w8[:, 3, g, :, ot * 128 : (ot + 1) * 128],
                        rhs=o_sb[:, g, :, :],
                        start=(g == 0),
                        stop=(g == 1),
                        perf_mode=DR,
                    )
                # out = (psum + pb') + x in one pass
                nc.vector.scalar_tensor_tensor(
                    out=ob4[:, ot, :],
                    in0=psm[:, 0, :],
                    scalar=pb[:, ot : ot + 1],
                    in1=x_sb[:, ot, isl],
                    op0=OP.add,
                    op1=OP.add,
                )
                if s == S - 1 and ib == IB - 1:
                    # tail: one DMA per ot, spread across queues so the
                    # descriptor issue doesn't serialize the drain
                    eng = (nc.sync, nc.scalar, nc.gpsimd, nc.sync)[ot]
                    eng.dma_start(
                        out=out_ap[s][:, ot : ot + 1, isl],
                        in_=ob4[:, ot : ot + 1, :],
                    )
            if not (s == S - 1 and ib == IB - 1):
                nc.sync.dma_start(out=out_ap[s][:, :, isl], in_=ob4[:])

        # ---- software-pipelined emission across the two samples ----
        emit_gn_alloc(0)
        emit_gn_stats_act(0, [0])   # t0 on ACT, in parallel with DVE
        emit_gn_stats(0, [1, 2])
        with tc.high_priority():
            emit_gn_chain(0, 0)
            emit_gn_apply(0, 0, ("v", "s"))  # t1's apply rides on ACT
        emit_gn_stats(0, [3])
        emit_waveb()
        with tc.high_priority():
            emit_gn_chain(0, 1)
            emit_gn_apply(0, 1, ("v", "s"))
        emit_qkv(0)
        emit_S(0, [0])          # s0 S phase, first i-block
        emit_gn_alloc(1)
        emit_gn_stats(1, [0, 1, 2, 3])  # s1 stats fill idle DVE here
        # high priority: slot the s1 chain/applies in as soon as their deps
        # are ready (the scheduler otherwise defers them until right before
        # qkv1 and the cross-engine ping-pong stalls the PE there)
        with tc.high_priority():
            emit_gn_chain(1, 0)
            emit_gn_chain(1, 1)
        emit_S(0, [1])          # s0 S phase, second i-block
        with tc.high_priority():
            emit_gn_apply(1, 0)
            emit_gn_apply(1, 1)
        emit_qkv(1)             # fills the PE while s0's exps drain
        emit_Z(0, 0)
        emit_S(1, [0])          # s1's first exps drain under attn2(0,0)
        emit_zb(0, 0)
        emit_attn2(0, 0)
        emit_Z(0, 1)
        emit_S(1, [1])
        emit_Z(1, 0)            # s1 Z heads + all remaining zb broadcasts
        emit_zb(0, 1)           # early: the tail phases then have no ACT
        emit_Z(1, 1)            # dependencies at all
        emit_zb(1, 0)
        emit_attn2(0, 1)
        emit_zb(1, 1)
        emit_attn2(1, 0)
        emit_attn2(1, 1)

    _split_excess_waits(nc)
    return nc


_NC = None


def kernel(x, norm_w, norm_b, qkv_w, qkv_b, proj_w, proj_b):
    global _NC, LAST_RESULT
    x = np.asarray(x, dtype=np.float32)
    norm_w = np.asarray(norm_w, dtype=np.float32)
    norm_b = np.asarray(norm_b, dtype=np.float32)
    qkv_w = np.asarray(qkv_w, dtype=np.float32)
    qkv_b = np.asarray(qkv_b, dtype=np.float32)
    proj_w = np.asarray(proj_w, dtype=np.float32)
    proj_b = np.asarray(proj_b, dtype=np.float32)

    # fold GroupNorm affine into qkv
    wq_full = qkv_w * norm_w[None, :]
    bq_full = qkv_b + qkv_w @ norm_b
    wq_, wk_, wv_ = wq_full[0:C], wq_full[C : 2 * C], wq_full[2 * C : 3 * C]
    bq_, bv_ = bq_full[0:C], bq_full[2 * C : 3 * C]
    pb_ = proj_w @ bv_ + proj_b

    def wtile(w):  # [o, c] -> DoubleRow lhsT planes [128, 2(g), 2(q), o]
        return w.T.reshape(2, 2, 128, C).transpose(2, 0, 1, 3)

    def btile(b):  # [C] -> [128, ct]
        return b.reshape(CT, 128).T

    wall = np.ascontiguousarray(
        np.stack(
            [wtile(wq_), wtile(wk_), wtile(wv_), wtile(proj_w)], axis=1
        ).reshape(128, 16, C).astype(ml_dtypes.float8_e4m3)
    )
    ball = np.ascontiguousarray(
        np.concatenate([btile(bq_), btile(pb_)], axis=1).astype(np.float32)
    )
    cl = np.arange(128)
    ghot = np.zeros((128, 2), np.float32)
    ghot[cl, cl // 64] = 1.0 / 64.0
    hhot = np.zeros((2, 128), np.float32)
    hhot[cl // 64, cl] = 1.0

    common = {
        "wall": wall,
        "ball": ball,
        "ca": ghot.astype(ml_dtypes.bfloat16),
        "cb": hhot.astype(ml_dtypes.bfloat16),
    }
    xr = np.ascontiguousarray(
        x.reshape(NCORES, S, C, N).astype(ml_dtypes.bfloat16)
    )
    in_maps = [dict(common, x=xr[i]) for i in range(NCORES)]

    if _NC is None:
        _NC = _build()
    res = run_bass_kernel_spmd(
        _NC, in_maps, core_ids=list(range(NCORES)), trace=TRACE
    )
    LAST_RESULT = res
    out = np.stack([res.results[i]["out"] for i in range(NCORES)])
    return np.ascontiguousarray(
        out.reshape(B, C, 32, 32).astype(np.float32)
    )
